# revision 14
# baseline (speedup 1.0000x reference)
"""BERT-base encoder (12L, B=4, S=2048, H=768) on 8 Trainium2 NeuronCores.

Sharding: 8 shards of 1024 tokens (sample b = core//2, seq-half = core%2).
Each core computes K/V for its own 1024 tokens, keeps them in SBUF, and
AllGathers them (two collectives: K then V) within its core pair. Attention
runs over the LOCAL half while the collective is in flight, then accumulates
the remote half, so the exchange is hidden behind compute.

On-chip layout: activations feature-major (h^T [768, 1024]); every projection
is an accumulated matmul with the stored [in, out] weights stationary.
LayerNorm weights of the producing LN are folded host-side into the consuming
projections (Wq/Wk/Wv get the previous ln2_w, Wi gets ln1_w), so the bf16
matmul operand h_bf holds the *pre-affine* normalized value; the f32 residual
stream h_m holds the affined value. Softmax: scores^T [k, q], exp on ScalarE,
ones-column appended to V so ctx PSUM row 64 accumulates sum(exp). All row
reciprocals/rsqrts use exp/ln (one ACT table set shared with attention's exp).
"""

import dataclasses

import numpy as np
import ml_dtypes

import concourse.bass as bass
import concourse.tile as tile
from concourse import bacc, mybir
from concourse.bass import IndirectOffsetOnAxis
from concourse.bass_utils import run_bass_kernel_spmd
from concourse.masks import make_identity
from concourse.alu_op_type import AluOpType

F32 = mybir.dt.float32
BF16 = mybir.dt.bfloat16
I32 = mybir.dt.int32
AF = mybir.ActivationFunctionType
BF = ml_dtypes.bfloat16


@dataclasses.dataclass
class Cfg:
    L: int = 12
    NH: int = 12
    DH: int = 64
    FF: int = 3072
    V: int = 30522
    TOK: int = 1024  # tokens per core
    S: int = 2048  # full sequence
    n_cores: int = 8
    EPS: float = 1e-12

    @property
    def Hd(self):
        return self.NH * self.DH

    @property
    def HC(self):
        return self.Hd // 128  # hidden chunks == head pairs

    @property
    def TB(self):
        return self.TOK // 128

    @property
    def KB(self):
        return self.S // 128

    @property
    def FC(self):
        return self.FF // 128

    @property
    def NHALF(self):
        return self.S // self.TOK  # cores per sample

    @property
    def QS(self):
        return [(q, min(512, self.TOK - q)) for q in range(0, self.TOK, 512)]


def build(cfg: Cfg, fake_cc: bool = False):
    L, NH, DH, FF, V = cfg.L, cfg.NH, cfg.DH, cfg.FF, cfg.V
    TOK, S, Hd = cfg.TOK, cfg.S, cfg.Hd
    HC, TB, KB, FC, NHALF = cfg.HC, cfg.TB, cfg.KB, cfg.FC, cfg.NHALF
    QS = cfg.QS
    TPH = TOK // 128  # k blocks per half
    VW = NH * (DH + 1)  # V row width incl. interleaved ones columns

    nc = bacc.Bacc(
        "TRN2",
        target_bir_lowering=False,
        debug=False,
        enable_asserts=True,
        num_devices=cfg.n_cores,
    )

    # ---------------- DRAM I/O ----------------
    wq_d = nc.dram_tensor("wq", [L, Hd, Hd], BF16, kind="ExternalInput").ap()
    wk_d = nc.dram_tensor("wk", [L, Hd, Hd], BF16, kind="ExternalInput").ap()
    wv_d = nc.dram_tensor("wv", [L, Hd, Hd], BF16, kind="ExternalInput").ap()
    wo_d = nc.dram_tensor("wo", [L, Hd, Hd], BF16, kind="ExternalInput").ap()
    wi_d = nc.dram_tensor("wi", [L, FC, 128, Hd], BF16, kind="ExternalInput").ap()
    wf_d = nc.dram_tensor("wf", [L, HC, 128, FF], BF16, kind="ExternalInput").ap()
    # packed per-layer params: bq|bk|bo|bf|l1w|l1b|l2w|l2b, each HC cols
    par_d = nc.dram_tensor("par", [L, 128, 8 * HC], F32, kind="ExternalInput").ap()
    bi_d = nc.dram_tensor("bi", [L, 128, FC], F32, kind="ExternalInput").ap()
    bv_d = nc.dram_tensor("bv", [L, 1, Hd], BF16, kind="ExternalInput").ap()
    we_d = nc.dram_tensor("wemb", [V, Hd], BF16, kind="ExternalInput").ap()
    pos_d = nc.dram_tensor("pos", [TOK, Hd], BF16, kind="ExternalInput").ap()
    epar_d = nc.dram_tensor("epar", [128, 2 * HC], F32, kind="ExternalInput").ap()
    ids_d = nc.dram_tensor("ids", [128, TB], I32, kind="ExternalInput").ap()
    mask_d = nc.dram_tensor("mask", [128, KB], F32, kind="ExternalInput").ap()
    kvidx_d = nc.dram_tensor("kvidx", [128, HC + TB], I32, kind="ExternalInput").ap()
    sel_d = nc.dram_tensor("sel", [NH, HC * 128], BF16, kind="ExternalInput").ap()
    y_d = nc.dram_tensor("y", [TOK, Hd], BF16, kind="ExternalOutput").ap()

    kvk_in = nc.dram_tensor("kvk_in", [Hd * TOK], BF16, kind="Internal").ap()
    kvk_out = nc.dram_tensor("kvk_out", [NHALF, Hd * TOK], BF16, kind="Internal").ap()
    kvv_in = nc.dram_tensor("kvv_in", [TOK * VW], BF16, kind="Internal").ap()
    kvv_out = nc.dram_tensor("kvv_out", [NHALF, TOK * VW], BF16, kind="Internal").ap()
    kvk_in_v = kvk_in.rearrange("(h t) -> h t", t=TOK)
    kvv_in_v = kvv_in.rearrange("(t w) -> t w", w=VW)
    kvk_rows = kvk_out.rearrange("n (h t) -> (n h) t", t=TOK)
    kvv_rows = kvv_out.rearrange("n (t w) -> (n t) w", w=VW)

    groups = [
        [g * NHALF + i for i in range(NHALF)] for g in range(cfg.n_cores // NHALF)
    ]

    with tile.TileContext(nc) as tc:
        # ---------------- persistent SBUF ----------------
        _frees = []

        def single(name, shape, dtype):
            t, fr = tc.tile(shape, dtype, name=name)
            _frees.append(fr)
            return t

        h_m = [single(f"h_m{i}", [128, TOK], F32) for i in range(HC)]
        lnin = [single(f"lnin{i}", [128, TOK], F32) for i in range(HC)]
        h_bf = [single(f"h_bf{i}", [128, TOK], BF16) for i in range(HC)]
        qT = [single(f"qT{i}", [128, TOK], BF16) for i in range(HC)]
        kT_loc = [single(f"kTl{i}", [128, TOK], BF16) for i in range(HC)]
        kT_rem = [single(f"kTr{i}", [128, TOK], BF16) for i in range(HC)]
        # v tiles padded to TOK wide so they can double as ffT slots
        v_sb = [single(f"v_sb{i}", [128, TOK], BF16) for i in range(KB)]
        ctxT = [single(f"ctxT{i}", [128, TOK], BF16) for i in range(HC)]
        den_all = single("den_all", [NH, TOK], BF16)
        rec_all = single("rec_all", [NH, TOK], BF16)

        # selector lhsT per head pair: col m<64 -> row 2hp, m>=64 -> row 2hp+1
        sel_all = single("sel_all", [NH, HC * 128], BF16)

        ids_sb = single("ids_sb", [128, TB], I32)
        mask_sb = single("mask_sb", [128, KB], F32)
        kvidx_sb = single("kvidx_sb", [128, HC + TB], I32)
        ones_c = single("ones_c", [128, 1], BF16)  # stats lhsT
        ones_r = single("ones_r", [1, 128], BF16)  # broadcast lhsT
        ident = single("ident", [128, 128], F32)
        identb = single("identb", [128, 128], BF16)
        eps_sb = single("eps_sb", [128, 1], F32)
        nc.vector.memset(eps_sb[:], cfg.EPS)
        nc.vector.memset(ones_c[:], 1.0)
        nc.vector.memset(ones_r[:], 1.0)
        make_identity(nc, ident[:])
        nc.vector.tensor_copy(identb[:], ident[:])
        nc.sync.dma_start(ids_sb[:], ids_d[:, :])
        nc.sync.dma_start(mask_sb[:], mask_d[:, :])
        nc.sync.dma_start(kvidx_sb[:], kvidx_d[:, :])
        nc.sync.dma_start(sel_all[:], sel_d[:, :])

        # ffT[oc] slots aliased onto kT_loc | kT_rem | v_sb[0:12]
        ffT_slots = kT_loc + kT_rem + v_sb[: FC - 2 * HC]
        assert len(ffT_slots) >= FC

        def ffT(oc):
            return ffT_slots[oc]

        with (
            tc.tile_pool(name="wp", bufs=9) as wp,
            tc.tile_pool(name="wfp", bufs=2) as wfp,
            tc.tile_pool(name="expp", bufs=3) as expp,
            tc.tile_pool(name="scr", bufs=2) as scr,
            tc.tile_pool(name="rows", bufs=2) as rows,
            tc.tile_pool(name="stg", bufs=2) as stg,
            tc.tile_pool(name="parp", bufs=2) as parp,
            tc.tile_pool(name="psA", bufs=2, space="PSUM") as psA,  # [128,TOK] f32: 2 banks x2
            tc.tile_pool(name="psB", bufs=2, space="PSUM") as psB,  # [128,TOK] f32: 2 banks x2
        ):
            # ---------------- embedding ----------------
            epar_t = parp.tile([128, 2 * HC], F32, tag="epar", name="epar")
            nc.sync.dma_start(epar_t[:], epar_d[:, :])
            bn_sub = 256 if Hd % 256 == 0 else 128
            nsub = Hd // bn_sub
            for tb in range(TB):
                emb_g = stg.tile([128, Hd], BF16, tag="vst", name=f"embg{tb}")
                nc.gpsimd.indirect_dma_start(
                    out=emb_g[:],
                    out_offset=None,
                    in_=we_d[:, :],
                    in_offset=IndirectOffsetOnAxis(ap=ids_sb[:, tb : tb + 1], axis=0),
                )
                pos_t = stg.tile([128, Hd], BF16, tag="vst", name=f"pos{tb}")
                nc.sync.dma_start(pos_t[:], pos_d[tb * 128 : (tb + 1) * 128, :])
                x = emb_g
                nc.vector.tensor_add(x[:], x[:], pos_t[:])
                stats = scr.tile([128, nsub, 6], F32, tag="bst", name=f"bst{tb}")
                for sgi in range(nsub):
                    nc.vector.bn_stats(
                        stats[:, sgi, :], x[:, sgi * bn_sub : (sgi + 1) * bn_sub]
                    )
                mv = scr.tile([128, 2], F32, tag="bmv", name=f"bmv{tb}")
                nc.vector.bn_aggr(mv[:], stats[:])
                # rstd = exp(-0.5*ln(var+eps))  (exp/ln table set)
                rstd = scr.tile([128, 1], F32, tag="brs", name=f"brs{tb}")
                nc.scalar.activation(rstd[:], mv[:, 1:2], AF.Ln, bias=eps_sb[:])
                nc.scalar.activation(rstd[:], rstd[:], AF.Exp, scale=-0.5)
                nc.vector.tensor_scalar(
                    x[:], x[:], mv[:, 0:1], rstd[:], AluOpType.subtract, AluOpType.mult
                )
                # transpose z to feature-major; h_bf = z, h_m = w*z + b
                for hc in range(HC):
                    pst = psB.tile([128, 128], BF16, tag="ctx", name=f"tp{tb}_{hc}")
                    nc.tensor.transpose(
                        pst[:], x[:, hc * 128 : (hc + 1) * 128], identb[:]
                    )
                    nc.vector.tensor_copy(h_bf[hc][:, tb * 128 : (tb + 1) * 128], pst[:])
                    nc.vector.tensor_scalar(
                        h_m[hc][:, tb * 128 : (tb + 1) * 128],
                        pst[:],
                        epar_t[:, hc : hc + 1],
                        epar_t[:, HC + hc : HC + hc + 1],
                        AluOpType.mult,
                        AluOpType.add,
                    )

            # ---------------- layers ----------------
            for l in range(L):
                par_t = parp.tile([128, 8 * HC], F32, tag="par", name=f"par{l}")
                nc.sync.dma_start(par_t[:], par_d[l])
                bi_t = parp.tile([128, FC], F32, tag="bi", name=f"bi{l}")
                nc.sync.dma_start(bi_t[:], bi_d[l])
                bv_t = parp.tile([1, Hd], BF16, tag="bv", bufs=1, name=f"bv{l}")
                nc.sync.dma_start(bv_t[:], bv_d[l])
                O_BQ, O_BK, O_BO, O_BF = 0, HC, 2 * HC, 3 * HC
                O_L1W, O_L1B, O_L2W, O_L2B = 4 * HC, 5 * HC, 6 * HC, 7 * HC

                # ---- K projection -> kT_loc + kv staging ----
                wk_t = [
                    wp.tile([128, Hd], BF16, tag="w", name=f"wk{l}_{ic}")
                    for ic in range(HC)
                ]
                for ic in range(HC):
                    nc.sync.dma_start(
                        wk_t[ic][:], wk_d[l, ic * 128 : (ic + 1) * 128, :]
                    )
                for hc in range(HC):
                    ps = psA.tile([128, TOK], F32, tag="sc", name=f"psk{l}_{hc}")
                    for q0, qn in QS:
                        for ic in range(HC):
                            nc.tensor.matmul(
                                ps[:, q0 : q0 + qn],
                                wk_t[ic][:, hc * 128 : (hc + 1) * 128],
                                h_bf[ic][:, q0 : q0 + qn],
                                start=(ic == 0),
                                stop=(ic == HC - 1),
                            )
                    nc.vector.tensor_scalar_add(
                        kT_loc[hc][:], ps[:], par_t[:, O_BK + hc : O_BK + hc + 1]
                    )
                    nc.sync.dma_start(
                        kvk_in_v[hc * 128 : (hc + 1) * 128, :], kT_loc[hc][:]
                    )

                if fake_cc:
                    for half in range(NHALF):
                        nc.sync.dma_start(kvk_out[half], kvk_in[:])
                else:
                    nc.gpsimd.collective_compute(
                        "AllGather",
                        AluOpType.bypass,
                        replica_groups=groups,
                        ins=[kvk_in[:].opt()],
                        outs=[kvk_out[:].opt()],
                    )

                # ---- V projection (token-major, bias via K=1 ones matmul) ----
                wv_t = [
                    wp.tile([128, Hd], BF16, tag="w", name=f"wv{l}_{ic}")
                    for ic in range(HC)
                ]
                for ic in range(HC):
                    nc.sync.dma_start(
                        wv_t[ic][:], wv_d[l, ic * 128 : (ic + 1) * 128, :]
                    )
                for tb in range(TB):
                    ps = psA.tile([128, TOK], F32, tag="sc", name=f"psv{l}_{tb}")
                    for n0 in range(0, Hd, 512):
                        nn = min(512, Hd - n0)
                        nc.tensor.matmul(
                            ps[:, n0 : n0 + nn],
                            ones_r[0:1, 0:128],
                            bv_t[0:1, n0 : n0 + nn],
                            start=True,
                            stop=False,
                        )
                        for ic in range(HC):
                            nc.tensor.matmul(
                                ps[:, n0 : n0 + nn],
                                h_bf[ic][:, tb * 128 : (tb + 1) * 128],
                                wv_t[ic][:, n0 : n0 + nn],
                                start=False,
                                stop=(ic == HC - 1),
                            )
                    vs = v_sb[tb]
                    vsv = vs[:, 0:VW].rearrange("p (n e) -> p n e", e=DH + 1)
                    nc.vector.tensor_copy(
                        vsv[:, :, 0:DH],
                        ps[:, 0:Hd].rearrange("p (n d) -> p n d", d=DH),
                    )
                    nc.vector.memset(vsv[:, :, DH : DH + 1], 1.0)
                    nc.sync.dma_start(kvv_in_v[tb * 128 : (tb + 1) * 128, :], vs[:, 0:VW])

                if fake_cc:
                    for half in range(NHALF):
                        nc.sync.dma_start(kvv_out[half], kvv_in[:])
                else:
                    nc.gpsimd.collective_compute(
                        "AllGather",
                        AluOpType.bypass,
                        replica_groups=groups,
                        ins=[kvv_in[:].opt()],
                        outs=[kvv_out[:].opt()],
                    )

                # ---- unpack remote K/V (indirect rows; waits on collectives) ----
                for hc in range(HC):
                    nc.gpsimd.indirect_dma_start(
                        out=kT_rem[hc][:],
                        out_offset=None,
                        in_=kvk_rows[:, :],
                        in_offset=IndirectOffsetOnAxis(
                            ap=kvidx_sb[:, hc : hc + 1], axis=0
                        ),
                    )
                for tb in range(TPH):
                    nc.gpsimd.indirect_dma_start(
                        out=v_sb[TPH + tb][:, 0:VW],
                        out_offset=None,
                        in_=kvv_rows[:, :],
                        in_offset=IndirectOffsetOnAxis(
                            ap=kvidx_sb[:, HC + tb : HC + tb + 1], axis=0
                        ),
                    )

                # ---- Q projection (overlaps the collectives) ----
                wq_t = [
                    wp.tile([128, Hd], BF16, tag="w", name=f"wq{l}_{ic}")
                    for ic in range(HC)
                ]
                for ic in range(HC):
                    nc.sync.dma_start(
                        wq_t[ic][:], wq_d[l, ic * 128 : (ic + 1) * 128, :]
                    )
                for hc in range(HC):
                    ps = psA.tile([128, TOK], F32, tag="sc", name=f"psq{l}_{hc}")
                    for q0, qn in QS:
                        for ic in range(HC):
                            nc.tensor.matmul(
                                ps[:, q0 : q0 + qn],
                                wq_t[ic][:, hc * 128 : (hc + 1) * 128],
                                h_bf[ic][:, q0 : q0 + qn],
                                start=(ic == 0),
                                stop=(ic == HC - 1),
                            )
                    nc.vector.tensor_scalar_add(
                        qT[hc][:], ps[:], par_t[:, O_BQ + hc : O_BQ + hc + 1]
                    )

                # ---- prefetch O-proj weights during attention ----
                wo_t = [
                    wp.tile([128, Hd], BF16, tag="w", name=f"wo{l}_{ic}")
                    for ic in range(HC)
                ]
                for ic in range(HC):
                    nc.sync.dma_start(
                        wo_t[ic][:], wo_d[l, ic * 128 : (ic + 1) * 128, :]
                    )

                # ---- attention: local kbs first (hide collectives), then remote ----
                for hp in range(HC):
                    ctx = [
                        psB.tile([65, TOK], F32, tag="ctx", name=f"ctx{l}_{hp}_{p}")
                        for p in range(2)
                    ]
                    for ki in range(KB):
                        kT_src = kT_loc[hp] if ki < TPH else kT_rem[hp]
                        tb = ki % TPH
                        sc = [
                            psA.tile(
                                [128, TOK], F32, tag="sc", name=f"sc{l}_{hp}_{ki}_{p}"
                            )
                            for p in range(2)
                        ]
                        # interleave the two 64-row halves: they co-execute
                        for q0, qn in QS:
                            for par_i in range(2):
                                b0 = 64 * par_i
                                nc.tensor.matmul(
                                    sc[par_i][:, q0 : q0 + qn],
                                    kT_src[b0 : b0 + 64, tb * 128 : (tb + 1) * 128],
                                    qT[hp][b0 : b0 + 64, q0 : q0 + qn],
                                    start=True,
                                    stop=True,
                                    tile_position=(b0, 0),
                                )
                        for par_i in range(2):
                            ex = expp.tile(
                                [128, TOK], BF16, tag="exp", name=f"ex{l}_{hp}_{ki}_{par_i}"
                            )
                            nc.scalar.activation(
                                ex[:],
                                sc[par_i][:],
                                AF.Exp,
                                bias=mask_sb[:, ki : ki + 1],
                                scale=float(1.0 / np.sqrt(DH)),
                            )
                            head = 2 * hp + par_i
                            for q0, qn in QS:
                                nc.tensor.matmul(
                                    ctx[par_i][:, q0 : q0 + qn],
                                    v_sb[ki][:, head * (DH + 1) : (head + 1) * (DH + 1)],
                                    ex[:, q0 : q0 + qn],
                                    start=(ki == 0),
                                    stop=(ki == KB - 1),
                                )
                    # drain ctx: unnormalized ctxT (bf16) + denominator rows
                    dstg = rows.tile([1, 2 * TOK], BF16, tag="dstg", name=f"ds{l}_{hp}")
                    for par_i in range(2):
                        b0 = 64 * par_i
                        nc.vector.tensor_copy(
                            ctxT[hp][b0 : b0 + 64, :], ctx[par_i][0:64, :]
                        )
                        nc.vector.tensor_copy(
                            dstg[0:1, par_i * TOK : (par_i + 1) * TOK],
                            ctx[par_i][64:65, :],
                        )
                    nc.sync.dma_start(
                        den_all[2 * hp : 2 * hp + 1, :], dstg[0:1, 0:TOK]
                    )
                    nc.sync.dma_start(
                        den_all[2 * hp + 1 : 2 * hp + 2, :], dstg[0:1, TOK : 2 * TOK]
                    )

                # reciprocal of all denominators at once: 1/d = exp(-ln(d))
                lnt = rows.tile([NH, TOK], F32, tag="t1", name=f"lnt{l}")
                nc.scalar.activation(lnt[:], den_all[:], AF.Ln)
                nc.scalar.activation(rec_all[:], lnt[:], AF.Exp, scale=-1.0)
                # normalize ctxT: selector matmul broadcasts both heads' reciprocal
                for hp in range(HC):
                    bc = psA.tile([128, TOK], F32, tag="sc", name=f"bc{l}_{hp}")
                    for q0, qn in QS:
                        nc.tensor.matmul(
                            bc[:, q0 : q0 + qn],
                            sel_all[:, hp * 128 : (hp + 1) * 128],
                            rec_all[:, q0 : q0 + qn],
                            start=True,
                            stop=True,
                        )
                    nc.vector.tensor_tensor(
                        ctxT[hp][:], ctxT[hp][:], bc[:], op=AluOpType.mult
                    )

                # ---- O projection + residual -> lnin ----
                for hc in range(HC):
                    ps = psA.tile([128, TOK], F32, tag="sc", name=f"pso{l}_{hc}")
                    for q0, qn in QS:
                        for ic in range(HC):
                            nc.tensor.matmul(
                                ps[:, q0 : q0 + qn],
                                wo_t[ic][:, hc * 128 : (hc + 1) * 128],
                                ctxT[ic][:, q0 : q0 + qn],
                                start=(ic == 0),
                                stop=(ic == HC - 1),
                            )
                    nc.vector.scalar_tensor_tensor(
                        lnin[hc][:],
                        ps[:],
                        par_t[:, O_BO + hc : O_BO + hc + 1],
                        h_m[hc][:],
                        op0=AluOpType.add,
                        op1=AluOpType.add,
                    )

                # ---- LN (feature-major): h_bf = z (pre-affine), h_m = w*z+b ----
                def layer_norm(w_off, b_off, tag):
                    for hc in range(HC):
                        nc.vector.tensor_copy(h_bf[hc][:], lnin[hc][:])
                        nc.vector.tensor_mul(qT[hc][:], lnin[hc][:], lnin[hc][:])
                    for qi, (q0, qn) in enumerate(QS):
                        s_ps = psB.tile([1, 512], F32, tag="ctx", name=f"sps{tag}{l}{qi}")
                        q_ps = psB.tile([1, 512], F32, tag="ctx", name=f"qps{tag}{l}{qi}")
                        for hc in range(HC):
                            nc.tensor.matmul(
                                s_ps[:, 0:qn],
                                ones_c[:],
                                h_bf[hc][:, q0 : q0 + qn],
                                start=(hc == 0),
                                stop=(hc == HC - 1),
                            )
                        for hc in range(HC):
                            nc.tensor.matmul(
                                q_ps[:, 0:qn],
                                ones_c[:],
                                qT[hc][:, q0 : q0 + qn],
                                start=(hc == 0),
                                stop=(hc == HC - 1),
                            )
                        # var*Hd = sumsq - sum^2/Hd ; rstd = exp(-0.5*ln(var+eps))
                        t1 = rows.tile([1, 512], F32, tag="t1", name=f"t1{tag}{l}{qi}")
                        nc.scalar.square(t1[:, 0:qn], s_ps[:, 0:qn])
                        nc.vector.scalar_tensor_tensor(
                            t1[:, 0:qn],
                            t1[:, 0:qn],
                            -1.0 / Hd,
                            q_ps[:, 0:qn],
                            op0=AluOpType.mult,
                            op1=AluOpType.add,
                        )
                        nc.scalar.activation(
                            t1[:, 0:qn], t1[:, 0:qn], AF.Ln, scale=1.0 / Hd,
                            bias=eps_sb[0:1, :],
                        )
                        mr = rows.tile([1, 1024], BF16, tag="mrb", name=f"mr{tag}{l}{qi}")
                        nc.scalar.activation(
                            mr[:, qn : 2 * qn], t1[:, 0:qn], AF.Exp, scale=-0.5
                        )
                        nc.vector.tensor_scalar_mul(mr[:, 0:qn], s_ps[:, 0:qn], 1.0 / Hd)
                        mbc = psA.tile([128, TOK], F32, tag="sc", name=f"mbc{tag}{l}_{q0}")
                        nc.tensor.matmul(
                            mbc[:, 0:qn],
                            ones_r[0:1, 0:128],
                            mr[0:1, 0:qn],
                            start=True,
                            stop=True,
                        )
                        nc.tensor.matmul(
                            mbc[:, qn : 2 * qn],
                            ones_r[0:1, 0:128],
                            mr[0:1, qn : 2 * qn],
                            start=True,
                            stop=True,
                        )
                        # stage broadcast in SBUF (bf16) so DVE reads run fast
                        mbs = expp.tile([128, TOK], BF16, tag="exp", name=f"mbs{tag}{l}{qi}")
                        nc.scalar.activation(mbs[:], mbc[:], AF.Copy)
                        for hc in range(HC):
                            t = scr.tile(
                                [128, 512], F32, tag="scr", name=f"sc{tag}{l}_{q0}_{hc}"
                            )
                            nc.vector.tensor_sub(
                                t[:, 0:qn], lnin[hc][:, q0 : q0 + qn], mbs[:, 0:qn]
                            )
                            nc.vector.tensor_tensor(
                                h_m[hc][:, q0 : q0 + qn],
                                t[:, 0:qn],
                                mbs[:, qn : 2 * qn],
                                op=AluOpType.mult,
                            )
                            nc.scalar.activation(
                                h_bf[hc][:, q0 : q0 + qn],
                                h_m[hc][:, q0 : q0 + qn],
                                AF.Copy,
                            )
                            nc.vector.tensor_scalar(
                                h_m[hc][:, q0 : q0 + qn],
                                h_m[hc][:, q0 : q0 + qn],
                                par_t[:, w_off + hc : w_off + hc + 1],
                                par_t[:, b_off + hc : b_off + hc + 1],
                                AluOpType.mult,
                                AluOpType.add,
                            )

                layer_norm(O_L1W, O_L1B, "a")  # h_bf = z1; h_m = attn (affined)

                # ---- FFN ----
                # phase 1: ff[oc] = gelu(Wi'^T z1 + bi') for full TOK per oc
                for oc in range(FC):
                    wi_t = wp.tile([128, Hd], BF16, tag="w", name=f"wi{l}_{oc}")
                    nc.sync.dma_start(wi_t[:], wi_d[l, oc])
                    ps = psA.tile([128, TOK], F32, tag="sc", name=f"psf{l}_{oc}")
                    for q0, qn in QS:
                        for ic in range(HC):
                            nc.tensor.matmul(
                                ps[:, q0 : q0 + qn],
                                wi_t[:, ic * 128 : (ic + 1) * 128],
                                h_bf[ic][:, q0 : q0 + qn],
                                start=(ic == 0),
                                stop=(ic == HC - 1),
                            )
                    nc.scalar.activation(
                        ffT(oc)[:], ps[:], AF.Gelu, bias=bi_t[:, oc : oc + 1]
                    )
                # phase 2: out[hc] = sum_fc Wf^T ff[fc]; + bf + attn -> lnin
                for hc in range(HC):
                    wf_t = wfp.tile([128, FF], BF16, tag="wf", name=f"wf{l}_{hc}")
                    nc.sync.dma_start(wf_t[:], wf_d[l, hc])
                    ps = psB.tile([128, TOK], F32, tag="ctx", name=f"psg{l}_{hc}")
                    for q0, qn in QS:
                        for fc in range(FC):
                            nc.tensor.matmul(
                                ps[:, q0 : q0 + qn],
                                wf_t[:, fc * 128 : (fc + 1) * 128],
                                ffT(fc)[:, q0 : q0 + qn],
                                start=(fc == 0),
                                stop=(fc == FC - 1),
                            )
                    nc.vector.scalar_tensor_tensor(
                        lnin[hc][:],
                        ps[:],
                        par_t[:, O_BF + hc : O_BF + hc + 1],
                        h_m[hc][:],
                        op0=AluOpType.add,
                        op1=AluOpType.add,
                    )

                layer_norm(O_L2W, O_L2B, "b")  # h_bf = z2; h_m = next-layer h

            # ------------- output (transpose back to token-major, bf16) -------------
            for tb in range(TB):
                ysb = stg.tile([128, Hd], BF16, tag="vst", name=f"ysb{tb}")
                for hc in range(HC):
                    pst = psB.tile([128, 128], F32, tag="ctx", name=f"yp{tb}_{hc}")
                    nc.tensor.transpose(
                        pst[:], h_m[hc][:, tb * 128 : (tb + 1) * 128], ident[:]
                    )
                    nc.vector.tensor_copy(ysb[:, hc * 128 : (hc + 1) * 128], pst[:])
                nc.sync.dma_start(y_d[tb * 128 : (tb + 1) * 128, :], ysb[:])

        for fr in reversed(_frees):
            fr()

    nc.compile()
    return nc


# ---------------------------------------------------------------------------
# host-side prep + execution
# ---------------------------------------------------------------------------


def _sel_matrix(cfg: Cfg):
    sel = np.zeros((cfg.NH, cfg.HC * 128), np.float32)
    for hp in range(cfg.HC):
        sel[2 * hp, hp * 128 : hp * 128 + 64] = 1.0
        sel[2 * hp + 1, hp * 128 + 64 : (hp + 1) * 128] = 1.0
    return sel.astype(BF)


def prep_shared_inputs(cfg: Cfg, d: dict) -> dict:
    """Inputs identical on every core (weights, with LN folding)."""
    L, Hd, FF, HC, FC = cfg.L, cfg.Hd, cfg.FF, cfg.HC, cfg.FC

    f32 = lambda x: np.asarray(x, np.float32)
    Wq, Wk, Wv, Wo = f32(d["Wq"]), f32(d["Wk"]), f32(d["Wv"]), f32(d["Wo"])
    Wi, Wf = f32(d["Wi"]), f32(d["Wf"])
    bq, bk, bv, bo = f32(d["bq"]), f32(d["bk"]), f32(d["bv"]), f32(d["bo"])
    bi, bf = f32(d["bi"]), f32(d["bf"])
    l1w, l1b = f32(d["ln1_w"]), f32(d["ln1_b"])
    l2w, l2b = f32(d["ln2_w"]), f32(d["ln2_b"])
    lew, leb = f32(d["ln_e_w"]), f32(d["ln_e_b"])

    # fold the producing LN's affine into consuming projections:
    # h = w*z + b  =>  h@W + c = z@(w[:,None]*W) + (b@W + c)
    w_pre = np.concatenate([lew[None], l2w[:-1]], axis=0)  # [L, Hd]
    b_pre = np.concatenate([leb[None], l2b[:-1]], axis=0)
    Wq_f = w_pre[:, :, None] * Wq
    Wk_f = w_pre[:, :, None] * Wk
    Wv_f = w_pre[:, :, None] * Wv
    bq_f = np.einsum("li,lio->lo", b_pre, Wq) + bq
    bk_f = np.einsum("li,lio->lo", b_pre, Wk) + bk
    bv_f = np.einsum("li,lio->lo", b_pre, Wv) + bv
    Wi_f = l1w[:, :, None] * Wi
    bi_f = np.einsum("li,lio->lo", l1b, Wi) + bi

    def colpack(x, n):  # [L, n*128] -> [L, 128, n]
        return np.ascontiguousarray(
            np.asarray(x, np.float32).reshape(L, n, 128).transpose(0, 2, 1)
        )

    par = np.concatenate(
        [
            colpack(bq_f, HC),
            colpack(bk_f, HC),
            colpack(bo, HC),
            colpack(bf, HC),
            colpack(l1w, HC),
            colpack(l1b, HC),
            colpack(l2w, HC),
            colpack(l2b, HC),
        ],
        axis=2,
    )
    # wi[l, oc, p, ic*128+j] = Wi[l, ic*128+p, oc*128+j]
    wi_r = np.ascontiguousarray(
        Wi_f.reshape(L, HC, 128, FC, 128)
        .transpose(0, 3, 2, 1, 4)
        .reshape(L, FC, 128, Hd)
        .astype(BF)
    )
    # wf[l, oc2, p, fc*128+j] = Wf[l, fc*128+p, oc2*128+j]
    wf_r = np.ascontiguousarray(
        Wf.reshape(L, FC, 128, HC, 128)
        .transpose(0, 3, 2, 1, 4)
        .reshape(L, HC, 128, FF)
        .astype(BF)
    )
    # epar[:, 0:HC] = lew cols, [:, HC:2HC] = leb cols
    epar = np.concatenate(
        [lew.reshape(HC, 128).T, leb.reshape(HC, 128).T], axis=1
    )
    return {
        "wq": Wq_f.astype(BF),
        "wk": Wk_f.astype(BF),
        "wv": Wv_f.astype(BF),
        "wo": Wo.astype(BF),
        "wi": wi_r,
        "wf": wf_r,
        "par": par,
        "bi": colpack(bi_f, FC),
        "bv": bv_f.astype(BF)[:, None, :],
        "wemb": f32(d["word_emb"]).astype(BF),
        "sel": _sel_matrix(cfg),
        "epar": np.ascontiguousarray(epar, np.float32),
    }


def prep_core_inputs(cfg: Cfg, core: int, d: dict, shared: dict) -> dict:
    TOK, TB, KB, HC, TPH = cfg.TOK, cfg.TB, cfg.KB, cfg.HC, cfg.TOK // 128
    b, hh = core // cfg.NHALF, core % cfg.NHALF
    ids = np.asarray(d["input_ids"], np.int32)[b, hh * TOK : (hh + 1) * TOK]
    mask = np.asarray(d["attention_mask"], np.float32)[b, 0, 0, :]
    pos = (
        np.asarray(d["pos_emb"], np.float32)[hh * TOK : (hh + 1) * TOK]
        + np.asarray(d["type_emb"], np.float32)[0][None, :]
    )
    m = dict(shared)
    m["pos"] = np.ascontiguousarray(pos).astype(BF)
    m["ids"] = np.ascontiguousarray(ids.reshape(TB, 128).T)
    # kb order: local half first, then remote half
    kb_order = [hh * TPH + j for j in range(TPH)] + [
        (1 - hh) * TPH + j for j in range(TPH)
    ]
    mask_cols = mask.reshape(KB, 128).T  # [128, KB] in global order
    m["mask"] = np.ascontiguousarray(mask_cols[:, kb_order])
    # indirect row indices for the remote half of kvk_out/kvv_out
    rem = 1 - hh
    ar = np.arange(128, dtype=np.int32)
    kcols = [rem * cfg.Hd + j * 128 + ar for j in range(HC)]
    vcols = [rem * TOK + j * 128 + ar for j in range(TPH)]
    m["kvidx"] = np.ascontiguousarray(np.stack(kcols + vcols, axis=1))
    return m


_CACHE: dict = {}


def kernel(**inputs) -> np.ndarray:
    cfg = Cfg()
    B = inputs["input_ids"].shape[0]
    if "nc" not in _CACHE:
        _CACHE["nc"] = build(cfg)
    nc = _CACHE["nc"]
    shared = prep_shared_inputs(cfg, inputs)
    in_maps = [prep_core_inputs(cfg, c, inputs, shared) for c in range(cfg.n_cores)]
    res = run_bass_kernel_spmd(nc, in_maps, core_ids=list(range(cfg.n_cores)))
    out = np.zeros((B, cfg.S, cfg.Hd), np.float32)
    for c in range(cfg.n_cores):
        b, hh = c // cfg.NHALF, c % cfg.NHALF
        out[b, hh * cfg.TOK : (hh + 1) * cfg.TOK, :] = np.asarray(
            res.results[c]["y"], np.float32
        )
    return out


# revision 16
# speedup vs baseline: 1.3683x; 1.3683x over previous
"""BERT-base encoder (12L, B=4, S=2048, H=768) on 8 Trainium2 NeuronCores.

Sharding: 8 shards of 1024 tokens (sample b = core//2, seq-half = core%2).
Each core computes K/V for its own 1024 tokens, keeps them in SBUF, and
AllGathers them (two collectives: K then V) within its core pair. Attention
runs over the LOCAL half while the collective is in flight, then accumulates
the remote half, so the exchange is hidden behind compute.

On-chip layout: activations feature-major (h^T [768, 1024]); every projection
is an accumulated matmul with the stored [in, out] weights stationary.
LayerNorm weights of the producing LN are folded host-side into the consuming
projections (Wq/Wk/Wv get the previous ln2_w, Wi gets ln1_w), so the bf16
matmul operand h_bf holds the *pre-affine* normalized value; the f32 residual
stream h_m holds the affined value. Softmax: scores^T [k, q], exp on ScalarE,
ones-column appended to V so ctx PSUM row 64 accumulates sum(exp). All row
reciprocals/rsqrts use exp/ln (one ACT table set shared with attention's exp).
"""

import dataclasses

import numpy as np
import ml_dtypes

import concourse.bass as bass
import concourse.tile as tile
from concourse import bacc, mybir
from concourse.bass import IndirectOffsetOnAxis
from concourse.bass_utils import run_bass_kernel_spmd
from concourse.masks import make_identity
from concourse.alu_op_type import AluOpType

F32 = mybir.dt.float32
BF16 = mybir.dt.bfloat16
I32 = mybir.dt.int32
AF = mybir.ActivationFunctionType
BF = ml_dtypes.bfloat16


@dataclasses.dataclass
class Cfg:
    L: int = 12
    NH: int = 12
    DH: int = 64
    FF: int = 3072
    V: int = 30522
    TOK: int = 1024  # tokens per core
    S: int = 2048  # full sequence
    n_cores: int = 8
    EPS: float = 1e-12

    @property
    def Hd(self):
        return self.NH * self.DH

    @property
    def HC(self):
        return self.Hd // 128  # hidden chunks == head pairs

    @property
    def TB(self):
        return self.TOK // 128

    @property
    def KB(self):
        return self.S // 128

    @property
    def FC(self):
        return self.FF // 128

    @property
    def NHALF(self):
        return self.S // self.TOK  # cores per sample

    @property
    def QS(self):
        return [(q, min(512, self.TOK - q)) for q in range(0, self.TOK, 512)]


def build(cfg: Cfg, fake_cc: bool = False):
    L, NH, DH, FF, V = cfg.L, cfg.NH, cfg.DH, cfg.FF, cfg.V
    TOK, S, Hd = cfg.TOK, cfg.S, cfg.Hd
    HC, TB, KB, FC, NHALF = cfg.HC, cfg.TB, cfg.KB, cfg.FC, cfg.NHALF
    QS = cfg.QS
    TPH = TOK // 128  # k blocks per half
    VW = NH * (DH + 1)  # V row width incl. interleaved ones columns

    nc = bacc.Bacc(
        "TRN2",
        target_bir_lowering=False,
        debug=False,
        enable_asserts=True,
        num_devices=cfg.n_cores,
    )

    # ---------------- DRAM I/O ----------------
    wq_d = nc.dram_tensor("wq", [L, Hd, Hd], BF16, kind="ExternalInput").ap()
    wk_d = nc.dram_tensor("wk", [L, Hd, Hd], BF16, kind="ExternalInput").ap()
    wv_d = nc.dram_tensor("wv", [L, Hd, Hd], BF16, kind="ExternalInput").ap()
    wo_d = nc.dram_tensor("wo", [L, Hd, Hd], BF16, kind="ExternalInput").ap()
    wi_d = nc.dram_tensor("wi", [L, FC, 128, Hd], BF16, kind="ExternalInput").ap()
    wf_d = nc.dram_tensor("wf", [L, HC, 128, FF], BF16, kind="ExternalInput").ap()
    # packed per-layer params: bq|bk|bo|bf|l1w|l1b|l2w|l2b, each HC cols
    par_d = nc.dram_tensor("par", [L, 128, 8 * HC], F32, kind="ExternalInput").ap()
    bi_d = nc.dram_tensor("bi", [L, 128, FC], F32, kind="ExternalInput").ap()
    bv_d = nc.dram_tensor("bv", [L, 1, Hd], BF16, kind="ExternalInput").ap()
    we_d = nc.dram_tensor("wemb", [V, Hd], BF16, kind="ExternalInput").ap()
    pos_d = nc.dram_tensor("pos", [TOK, Hd], BF16, kind="ExternalInput").ap()
    epar_d = nc.dram_tensor("epar", [128, 2 * HC], F32, kind="ExternalInput").ap()
    ids_d = nc.dram_tensor("ids", [128, TB], I32, kind="ExternalInput").ap()
    mask_d = nc.dram_tensor("mask", [128, KB], F32, kind="ExternalInput").ap()
    kvidx_d = nc.dram_tensor("kvidx", [128, HC + TB], I32, kind="ExternalInput").ap()
    sel_d = nc.dram_tensor("sel", [NH, HC * 128], BF16, kind="ExternalInput").ap()
    y_d = nc.dram_tensor("y", [TOK, Hd], BF16, kind="ExternalOutput").ap()

    kvk_in = nc.dram_tensor("kvk_in", [Hd * TOK], BF16, kind="Internal").ap()
    kvk_out = nc.dram_tensor("kvk_out", [NHALF, Hd * TOK], BF16, kind="Internal").ap()
    kvv_in = nc.dram_tensor("kvv_in", [TOK * VW], BF16, kind="Internal").ap()
    kvv_out = nc.dram_tensor("kvv_out", [NHALF, TOK * VW], BF16, kind="Internal").ap()
    kvk_in_v = kvk_in.rearrange("(h t) -> h t", t=TOK)
    kvv_in_v = kvv_in.rearrange("(t w) -> t w", w=VW)
    kvk_rows = kvk_out.rearrange("n (h t) -> (n h) t", t=TOK)
    kvv_rows = kvv_out.rearrange("n (t w) -> (n t) w", w=VW)

    groups = [
        [g * NHALF + i for i in range(NHALF)] for g in range(cfg.n_cores // NHALF)
    ]

    with tile.TileContext(nc) as tc:
        # ---------------- persistent SBUF ----------------
        _frees = []

        def single(name, shape, dtype):
            t, fr = tc.tile(shape, dtype, name=name)
            _frees.append(fr)
            return t

        h_m = [single(f"h_m{i}", [128, TOK], F32) for i in range(HC)]
        lnin = [single(f"lnin{i}", [128, TOK], F32) for i in range(HC)]
        h_bf = [single(f"h_bf{i}", [128, TOK], BF16) for i in range(HC)]
        qT = [single(f"qT{i}", [128, TOK], BF16) for i in range(HC)]
        kT_loc = [single(f"kTl{i}", [128, TOK], BF16) for i in range(HC)]
        kT_rem = [single(f"kTr{i}", [128, TOK], BF16) for i in range(HC)]
        # v tiles padded to TOK wide so they can double as ffT slots
        v_sb = [single(f"v_sb{i}", [128, TOK], BF16) for i in range(KB)]
        ctxT = [single(f"ctxT{i}", [128, TOK], BF16) for i in range(HC)]
        den_all = single("den_all", [NH, TOK], BF16)
        rec_all = single("rec_all", [NH, TOK], BF16)

        # selector lhsT per head pair: col m<64 -> row 2hp, m>=64 -> row 2hp+1
        sel_all = single("sel_all", [NH, HC * 128], BF16)

        ids_sb = single("ids_sb", [128, TB], I32)
        mask_sb = single("mask_sb", [128, KB], F32)
        kvidx_sb = single("kvidx_sb", [128, HC + TB], I32)
        ones_c = single("ones_c", [128, 1], BF16)  # stats lhsT
        ones_r = single("ones_r", [1, 128], BF16)  # broadcast lhsT
        ident = single("ident", [128, 128], F32)
        identb = single("identb", [128, 128], BF16)
        eps_sb = single("eps_sb", [128, 1], F32)
        nc.vector.memset(eps_sb[:], cfg.EPS)
        nc.vector.memset(ones_c[:], 1.0)
        nc.vector.memset(ones_r[:], 1.0)
        make_identity(nc, ident[:])
        nc.vector.tensor_copy(identb[:], ident[:])
        nc.sync.dma_start(ids_sb[:], ids_d[:, :])
        nc.sync.dma_start(mask_sb[:], mask_d[:, :])
        nc.sync.dma_start(kvidx_sb[:], kvidx_d[:, :])
        nc.sync.dma_start(sel_all[:], sel_d[:, :])

        # ffT[oc] slots aliased onto kT_loc | kT_rem | v_sb[0:12]
        ffT_slots = kT_loc + kT_rem + v_sb[: FC - 2 * HC]
        assert len(ffT_slots) >= FC

        def ffT(oc):
            return ffT_slots[oc]

        with (
            tc.tile_pool(name="wp", bufs=8) as wp,
            tc.tile_pool(name="wfp", bufs=3) as wfp,
            tc.tile_pool(name="expp", bufs=3) as expp,
            tc.tile_pool(name="scr", bufs=2) as scr,
            tc.tile_pool(name="rows", bufs=2) as rows,
            tc.tile_pool(name="stg", bufs=2) as stg,
            tc.tile_pool(name="parp", bufs=2) as parp,
            tc.tile_pool(name="psA", bufs=2, space="PSUM") as psA,  # [128,TOK] f32: 2 banks x2
            tc.tile_pool(name="psB", bufs=2, space="PSUM") as psB,  # [128,TOK] f32: 2 banks x2
        ):
            # ---------------- embedding ----------------
            epar_t = parp.tile([128, 2 * HC], F32, tag="epar", name="epar")
            nc.sync.dma_start(epar_t[:], epar_d[:, :])
            bn_sub = 256 if Hd % 256 == 0 else 128
            nsub = Hd // bn_sub
            for tb in range(TB):
                emb_g = stg.tile([128, Hd], BF16, tag="vst", name=f"embg{tb}")
                nc.gpsimd.indirect_dma_start(
                    out=emb_g[:],
                    out_offset=None,
                    in_=we_d[:, :],
                    in_offset=IndirectOffsetOnAxis(ap=ids_sb[:, tb : tb + 1], axis=0),
                )
                pos_t = stg.tile([128, Hd], BF16, tag="vst", name=f"pos{tb}")
                nc.sync.dma_start(pos_t[:], pos_d[tb * 128 : (tb + 1) * 128, :])
                x = emb_g
                nc.vector.tensor_add(x[:], x[:], pos_t[:])
                stats = scr.tile([128, nsub, 6], F32, tag="bst", name=f"bst{tb}")
                for sgi in range(nsub):
                    nc.vector.bn_stats(
                        stats[:, sgi, :], x[:, sgi * bn_sub : (sgi + 1) * bn_sub]
                    )
                mv = scr.tile([128, 2], F32, tag="bmv", name=f"bmv{tb}")
                nc.vector.bn_aggr(mv[:], stats[:])
                # rstd = exp(-0.5*ln(var+eps))  (exp/ln table set)
                rstd = scr.tile([128, 1], F32, tag="brs", name=f"brs{tb}")
                nc.scalar.activation(rstd[:], mv[:, 1:2], AF.Ln, bias=eps_sb[:])
                nc.scalar.activation(rstd[:], rstd[:], AF.Exp, scale=-0.5)
                nc.vector.tensor_scalar(
                    x[:], x[:], mv[:, 0:1], rstd[:], AluOpType.subtract, AluOpType.mult
                )
                # transpose z to feature-major; h_bf = z, h_m = w*z + b
                for hc in range(HC):
                    pst = psB.tile([128, 128], BF16, tag="ctx", name=f"tp{tb}_{hc}")
                    nc.tensor.transpose(
                        pst[:], x[:, hc * 128 : (hc + 1) * 128], identb[:]
                    )
                    nc.vector.tensor_copy(h_bf[hc][:, tb * 128 : (tb + 1) * 128], pst[:])
                    nc.vector.tensor_scalar(
                        h_m[hc][:, tb * 128 : (tb + 1) * 128],
                        pst[:],
                        epar_t[:, hc : hc + 1],
                        epar_t[:, HC + hc : HC + hc + 1],
                        AluOpType.mult,
                        AluOpType.add,
                    )

            # ---------------- layers ----------------
            for l in range(L):
                par_t = parp.tile([128, 8 * HC], F32, tag="par", name=f"par{l}")
                nc.sync.dma_start(par_t[:], par_d[l])
                bi_t = parp.tile([128, FC], F32, tag="bi", name=f"bi{l}")
                nc.sync.dma_start(bi_t[:], bi_d[l])
                bv_t = parp.tile([1, Hd], BF16, tag="bv", bufs=1, name=f"bv{l}")
                nc.sync.dma_start(bv_t[:], bv_d[l])
                O_BQ, O_BK, O_BO, O_BF = 0, HC, 2 * HC, 3 * HC
                O_L1W, O_L1B, O_L2W, O_L2B = 4 * HC, 5 * HC, 6 * HC, 7 * HC

                # ---- K projection -> kT_loc + kv staging ----
                wk_t = [
                    wp.tile([128, Hd], BF16, tag="w", name=f"wk{l}_{ic}")
                    for ic in range(HC)
                ]
                for ic in range(HC):
                    nc.sync.dma_start(
                        wk_t[ic][:], wk_d[l, ic * 128 : (ic + 1) * 128, :]
                    )
                for hc in range(HC):
                    ps = psA.tile([128, TOK], F32, tag="sc", name=f"psk{l}_{hc}")
                    for q0, qn in QS:
                        for ic in range(HC):
                            nc.tensor.matmul(
                                ps[:, q0 : q0 + qn],
                                wk_t[ic][:, hc * 128 : (hc + 1) * 128],
                                h_bf[ic][:, q0 : q0 + qn],
                                start=(ic == 0),
                                stop=(ic == HC - 1),
                            )
                    nc.vector.tensor_scalar_add(
                        kT_loc[hc][:], ps[:], par_t[:, O_BK + hc : O_BK + hc + 1]
                    )
                    nc.sync.dma_start(
                        kvk_in_v[hc * 128 : (hc + 1) * 128, :], kT_loc[hc][:]
                    )

                if fake_cc:
                    for half in range(NHALF):
                        nc.sync.dma_start(kvk_out[half], kvk_in[:])
                else:
                    nc.gpsimd.collective_compute(
                        "AllGather",
                        AluOpType.bypass,
                        replica_groups=groups,
                        ins=[kvk_in[:].opt()],
                        outs=[kvk_out[:].opt()],
                    )

                # ---- V projection (token-major, bias via K=1 ones matmul) ----
                wv_t = [
                    wp.tile([128, Hd], BF16, tag="w", name=f"wv{l}_{ic}")
                    for ic in range(HC)
                ]
                for ic in range(HC):
                    nc.sync.dma_start(
                        wv_t[ic][:], wv_d[l, ic * 128 : (ic + 1) * 128, :]
                    )
                for tb in range(TB):
                    ps = psA.tile([128, TOK], F32, tag="sc", name=f"psv{l}_{tb}")
                    for n0 in range(0, Hd, 512):
                        nn = min(512, Hd - n0)
                        nc.tensor.matmul(
                            ps[:, n0 : n0 + nn],
                            ones_r[0:1, 0:128],
                            bv_t[0:1, n0 : n0 + nn],
                            start=True,
                            stop=False,
                        )
                        for ic in range(HC):
                            nc.tensor.matmul(
                                ps[:, n0 : n0 + nn],
                                h_bf[ic][:, tb * 128 : (tb + 1) * 128],
                                wv_t[ic][:, n0 : n0 + nn],
                                start=False,
                                stop=(ic == HC - 1),
                            )
                    vs = v_sb[tb]
                    vsv = vs[:, 0:VW].rearrange("p (n e) -> p n e", e=DH + 1)
                    nc.vector.tensor_copy(
                        vsv[:, :, 0:DH],
                        ps[:, 0:Hd].rearrange("p (n d) -> p n d", d=DH),
                    )
                    nc.vector.memset(vsv[:, :, DH : DH + 1], 1.0)
                    nc.sync.dma_start(kvv_in_v[tb * 128 : (tb + 1) * 128, :], vs[:, 0:VW])

                if fake_cc:
                    for half in range(NHALF):
                        nc.sync.dma_start(kvv_out[half], kvv_in[:])
                else:
                    nc.gpsimd.collective_compute(
                        "AllGather",
                        AluOpType.bypass,
                        replica_groups=groups,
                        ins=[kvv_in[:].opt()],
                        outs=[kvv_out[:].opt()],
                    )

                # ---- unpack remote K/V (indirect rows; waits on collectives) ----
                for hc in range(HC):
                    nc.gpsimd.indirect_dma_start(
                        out=kT_rem[hc][:],
                        out_offset=None,
                        in_=kvk_rows[:, :],
                        in_offset=IndirectOffsetOnAxis(
                            ap=kvidx_sb[:, hc : hc + 1], axis=0
                        ),
                    )
                for tb in range(TPH):
                    nc.gpsimd.indirect_dma_start(
                        out=v_sb[TPH + tb][:, 0:VW],
                        out_offset=None,
                        in_=kvv_rows[:, :],
                        in_offset=IndirectOffsetOnAxis(
                            ap=kvidx_sb[:, HC + tb : HC + tb + 1], axis=0
                        ),
                    )

                # ---- Q projection (overlaps the collectives) ----
                wq_t = [
                    wp.tile([128, Hd], BF16, tag="w", name=f"wq{l}_{ic}")
                    for ic in range(HC)
                ]
                for ic in range(HC):
                    nc.sync.dma_start(
                        wq_t[ic][:], wq_d[l, ic * 128 : (ic + 1) * 128, :]
                    )
                for hc in range(HC):
                    ps = psA.tile([128, TOK], F32, tag="sc", name=f"psq{l}_{hc}")
                    for q0, qn in QS:
                        for ic in range(HC):
                            nc.tensor.matmul(
                                ps[:, q0 : q0 + qn],
                                wq_t[ic][:, hc * 128 : (hc + 1) * 128],
                                h_bf[ic][:, q0 : q0 + qn],
                                start=(ic == 0),
                                stop=(ic == HC - 1),
                            )
                    nc.vector.tensor_scalar_add(
                        qT[hc][:], ps[:], par_t[:, O_BQ + hc : O_BQ + hc + 1]
                    )

                # ---- prefetch O-proj weights during attention ----
                wo_t = [
                    wp.tile([128, Hd], BF16, tag="w", name=f"wo{l}_{ic}")
                    for ic in range(HC)
                ]
                for ic in range(HC):
                    nc.sync.dma_start(
                        wo_t[ic][:], wo_d[l, ic * 128 : (ic + 1) * 128, :]
                    )

                # ---- attention: local kbs first (hide collectives), then remote ----
                for hp in range(HC):
                    ctx = [
                        psB.tile([65, TOK], F32, tag="ctx", name=f"ctx{l}_{hp}_{p}")
                        for p in range(2)
                    ]
                    for ki in range(KB):
                        kT_src = kT_loc[hp] if ki < TPH else kT_rem[hp]
                        tb = ki % TPH
                        sc = [
                            psA.tile(
                                [128, TOK], F32, tag="sc", name=f"sc{l}_{hp}_{ki}_{p}"
                            )
                            for p in range(2)
                        ]
                        # interleave the two 64-row halves: they co-execute
                        for q0, qn in QS:
                            for par_i in range(2):
                                b0 = 64 * par_i
                                nc.tensor.matmul(
                                    sc[par_i][:, q0 : q0 + qn],
                                    kT_src[b0 : b0 + 64, tb * 128 : (tb + 1) * 128],
                                    qT[hp][b0 : b0 + 64, q0 : q0 + qn],
                                    start=True,
                                    stop=True,
                                    tile_position=(b0, 0),
                                )
                        for par_i in range(2):
                            ex = expp.tile(
                                [128, TOK], BF16, tag="exp", name=f"ex{l}_{hp}_{ki}_{par_i}"
                            )
                            nc.scalar.activation(
                                ex[:],
                                sc[par_i][:],
                                AF.Exp,
                                bias=mask_sb[:, ki : ki + 1],
                                scale=float(1.0 / np.sqrt(DH)),
                            )
                            head = 2 * hp + par_i
                            for q0, qn in QS:
                                nc.tensor.matmul(
                                    ctx[par_i][:, q0 : q0 + qn],
                                    v_sb[ki][:, head * (DH + 1) : (head + 1) * (DH + 1)],
                                    ex[:, q0 : q0 + qn],
                                    start=(ki == 0),
                                    stop=(ki == KB - 1),
                                )
                    # drain ctx: unnormalized ctxT (bf16) + denominator rows
                    dstg = rows.tile([1, 2 * TOK], BF16, tag="dstg", name=f"ds{l}_{hp}")
                    for par_i in range(2):
                        b0 = 64 * par_i
                        nc.vector.tensor_copy(
                            ctxT[hp][b0 : b0 + 64, :], ctx[par_i][0:64, :]
                        )
                        nc.vector.tensor_copy(
                            dstg[0:1, par_i * TOK : (par_i + 1) * TOK],
                            ctx[par_i][64:65, :],
                        )
                    nc.sync.dma_start(
                        den_all[2 * hp : 2 * hp + 1, :], dstg[0:1, 0:TOK]
                    )
                    nc.sync.dma_start(
                        den_all[2 * hp + 1 : 2 * hp + 2, :], dstg[0:1, TOK : 2 * TOK]
                    )

                # reciprocal of all denominators at once: 1/d = exp(-ln(d))
                lnt = rows.tile([NH, TOK], F32, tag="t1", name=f"lnt{l}")
                nc.scalar.activation(lnt[:], den_all[:], AF.Ln)
                nc.scalar.activation(rec_all[:], lnt[:], AF.Exp, scale=-1.0)
                # normalize ctxT: selector matmul broadcasts both heads' reciprocal
                for hp in range(HC):
                    bc = psA.tile([128, TOK], F32, tag="sc", name=f"bc{l}_{hp}")
                    for q0, qn in QS:
                        nc.tensor.matmul(
                            bc[:, q0 : q0 + qn],
                            sel_all[:, hp * 128 : (hp + 1) * 128],
                            rec_all[:, q0 : q0 + qn],
                            start=True,
                            stop=True,
                        )
                    nc.vector.tensor_tensor(
                        ctxT[hp][:], ctxT[hp][:], bc[:], op=AluOpType.mult
                    )

                # ---- O projection + residual -> lnin ----
                for hc in range(HC):
                    ps = psA.tile([128, TOK], F32, tag="sc", name=f"pso{l}_{hc}")
                    for q0, qn in QS:
                        for ic in range(HC):
                            nc.tensor.matmul(
                                ps[:, q0 : q0 + qn],
                                wo_t[ic][:, hc * 128 : (hc + 1) * 128],
                                ctxT[ic][:, q0 : q0 + qn],
                                start=(ic == 0),
                                stop=(ic == HC - 1),
                            )
                    nc.vector.scalar_tensor_tensor(
                        lnin[hc][:],
                        ps[:],
                        par_t[:, O_BO + hc : O_BO + hc + 1],
                        h_m[hc][:],
                        op0=AluOpType.add,
                        op1=AluOpType.add,
                    )

                # ---- LN (feature-major): h_bf = z (pre-affine), h_m = w*z+b ----
                def layer_norm(w_off, b_off, tag):
                    # ctxT doubles as the bf16 cast of lnin (stats + fast path)
                    for hc in range(HC):
                        nc.vector.tensor_copy(ctxT[hc][:], lnin[hc][:])
                        nc.vector.tensor_mul(qT[hc][:], ctxT[hc][:], ctxT[hc][:])
                    mbs_l = []
                    for qi, (q0, qn) in enumerate(QS):
                        s_ps = psB.tile([1, 512], F32, tag="ctx", name=f"sps{tag}{l}{qi}")
                        q_ps = psB.tile([1, 512], F32, tag="ctx", name=f"qps{tag}{l}{qi}")
                        for hc in range(HC):
                            nc.tensor.matmul(
                                s_ps[:, 0:qn],
                                ones_c[:],
                                ctxT[hc][:, q0 : q0 + qn],
                                start=(hc == 0),
                                stop=(hc == HC - 1),
                            )
                        for hc in range(HC):
                            nc.tensor.matmul(
                                q_ps[:, 0:qn],
                                ones_c[:],
                                qT[hc][:, q0 : q0 + qn],
                                start=(hc == 0),
                                stop=(hc == HC - 1),
                            )
                        # var*Hd = sumsq - sum^2/Hd ; rstd = exp(-0.5*ln(var+eps))
                        t1 = rows.tile([1, 512], F32, tag="t1", name=f"t1{tag}{l}{qi}")
                        nc.scalar.square(t1[:, 0:qn], s_ps[:, 0:qn])
                        nc.vector.scalar_tensor_tensor(
                            t1[:, 0:qn],
                            t1[:, 0:qn],
                            -1.0 / Hd,
                            q_ps[:, 0:qn],
                            op0=AluOpType.mult,
                            op1=AluOpType.add,
                        )
                        nc.scalar.activation(
                            t1[:, 0:qn], t1[:, 0:qn], AF.Ln, scale=1.0 / Hd,
                            bias=eps_sb[0:1, :],
                        )
                        mr = rows.tile([1, 1024], BF16, tag="mrb", name=f"mr{tag}{l}{qi}")
                        nc.scalar.activation(
                            mr[:, qn : 2 * qn], t1[:, 0:qn], AF.Exp, scale=-0.5
                        )
                        nc.vector.tensor_scalar_mul(mr[:, 0:qn], s_ps[:, 0:qn], 1.0 / Hd)
                        mbc = psA.tile([128, TOK], F32, tag="sc", name=f"mbc{tag}{l}_{q0}")
                        nc.tensor.matmul(
                            mbc[:, 0:qn],
                            ones_r[0:1, 0:128],
                            mr[0:1, 0:qn],
                            start=True,
                            stop=True,
                        )
                        nc.tensor.matmul(
                            mbc[:, qn : 2 * qn],
                            ones_r[0:1, 0:128],
                            mr[0:1, qn : 2 * qn],
                            start=True,
                            stop=True,
                        )
                        # stage broadcast in SBUF (bf16); frees the PSUM slot fast
                        mbs = expp.tile([128, TOK], BF16, tag="exp", name=f"mbs{tag}{l}{qi}")
                        nc.scalar.activation(mbs[:], mbc[:], AF.Copy)
                        mbs_l.append(mbs)
                        # FAST path: h_bf = (x - mu) * rstd, all bf16 (FFN waits on this)
                        for hc in range(HC):
                            tb = scr.tile(
                                [128, 512], BF16, tag="scb", name=f"tb{tag}{l}_{q0}_{hc}"
                            )
                            nc.vector.tensor_sub(
                                tb[:, 0:qn], ctxT[hc][:, q0 : q0 + qn], mbs[:, 0:qn]
                            )
                            nc.vector.tensor_tensor(
                                h_bf[hc][:, q0 : q0 + qn],
                                tb[:, 0:qn],
                                mbs[:, qn : 2 * qn],
                                op=AluOpType.mult,
                            )
                    # SLOW path (overlaps FFN): f32 h_m = w*(x-mu)*rstd + b
                    for qi, (q0, qn) in enumerate(QS):
                        mbs = mbs_l[qi]
                        for hc in range(HC):
                            t = scr.tile(
                                [128, 512], F32, tag="scr", name=f"sm{tag}{l}_{q0}_{hc}"
                            )
                            nc.vector.tensor_sub(
                                t[:, 0:qn], lnin[hc][:, q0 : q0 + qn], mbs[:, 0:qn]
                            )
                            nc.vector.tensor_tensor(
                                h_m[hc][:, q0 : q0 + qn],
                                t[:, 0:qn],
                                mbs[:, qn : 2 * qn],
                                op=AluOpType.mult,
                            )
                            nc.vector.tensor_scalar(
                                h_m[hc][:, q0 : q0 + qn],
                                h_m[hc][:, q0 : q0 + qn],
                                par_t[:, w_off + hc : w_off + hc + 1],
                                par_t[:, b_off + hc : b_off + hc + 1],
                                AluOpType.mult,
                                AluOpType.add,
                            )

                layer_norm(O_L1W, O_L1B, "a")  # h_bf = z1; h_m = attn (affined)

                # ---- FFN ----
                # phase 1: ff[oc] = gelu(Wi'^T z1 + bi') for full TOK per oc
                for oc in range(FC):
                    wi_t = wp.tile([128, Hd], BF16, tag="w", name=f"wi{l}_{oc}")
                    nc.sync.dma_start(wi_t[:], wi_d[l, oc])
                    ps = psA.tile([128, TOK], F32, tag="sc", name=f"psf{l}_{oc}")
                    for q0, qn in QS:
                        for ic in range(HC):
                            nc.tensor.matmul(
                                ps[:, q0 : q0 + qn],
                                wi_t[:, ic * 128 : (ic + 1) * 128],
                                h_bf[ic][:, q0 : q0 + qn],
                                start=(ic == 0),
                                stop=(ic == HC - 1),
                            )
                    nc.scalar.activation(
                        ffT(oc)[:], ps[:], AF.Gelu, bias=bi_t[:, oc : oc + 1]
                    )
                # phase 2: out[hc] = sum_fc Wf^T ff[fc]; + bf + attn -> lnin
                FH = FF // 2
                for hc in range(HC):
                    wf_h = [
                        wfp.tile([128, FH], BF16, tag="wf", name=f"wf{l}_{hc}_{h}")
                        for h in range(2)
                    ]
                    for h in range(2):
                        nc.sync.dma_start(wf_h[h][:], wf_d[l, hc, :, h * FH : (h + 1) * FH])
                    ps = psB.tile([128, TOK], F32, tag="ctx", name=f"psg{l}_{hc}")
                    for q0, qn in QS:
                        for fc in range(FC):
                            wsrc = wf_h[fc // (FC // 2)]
                            nc.tensor.matmul(
                                ps[:, q0 : q0 + qn],
                                wsrc[:, (fc % (FC // 2)) * 128 : (fc % (FC // 2) + 1) * 128],
                                ffT(fc)[:, q0 : q0 + qn],
                                start=(fc == 0),
                                stop=(fc == FC - 1),
                            )
                    nc.vector.scalar_tensor_tensor(
                        lnin[hc][:],
                        ps[:],
                        par_t[:, O_BF + hc : O_BF + hc + 1],
                        h_m[hc][:],
                        op0=AluOpType.add,
                        op1=AluOpType.add,
                    )

                layer_norm(O_L2W, O_L2B, "b")  # h_bf = z2; h_m = next-layer h

            # ------------- output (transpose back to token-major, bf16) -------------
            for tb in range(TB):
                ysb = stg.tile([128, Hd], BF16, tag="vst", name=f"ysb{tb}")
                for hc in range(HC):
                    pst = psB.tile([128, 128], F32, tag="ctx", name=f"yp{tb}_{hc}")
                    nc.tensor.transpose(
                        pst[:], h_m[hc][:, tb * 128 : (tb + 1) * 128], ident[:]
                    )
                    nc.vector.tensor_copy(ysb[:, hc * 128 : (hc + 1) * 128], pst[:])
                nc.sync.dma_start(y_d[tb * 128 : (tb + 1) * 128, :], ysb[:])

        for fr in reversed(_frees):
            fr()

    nc.compile()
    return nc


# ---------------------------------------------------------------------------
# host-side prep + execution
# ---------------------------------------------------------------------------


def _sel_matrix(cfg: Cfg):
    sel = np.zeros((cfg.NH, cfg.HC * 128), np.float32)
    for hp in range(cfg.HC):
        sel[2 * hp, hp * 128 : hp * 128 + 64] = 1.0
        sel[2 * hp + 1, hp * 128 + 64 : (hp + 1) * 128] = 1.0
    return sel.astype(BF)


def prep_shared_inputs(cfg: Cfg, d: dict) -> dict:
    """Inputs identical on every core (weights, with LN folding)."""
    L, Hd, FF, HC, FC = cfg.L, cfg.Hd, cfg.FF, cfg.HC, cfg.FC

    f32 = lambda x: np.asarray(x, np.float32)
    Wq, Wk, Wv, Wo = f32(d["Wq"]), f32(d["Wk"]), f32(d["Wv"]), f32(d["Wo"])
    Wi, Wf = f32(d["Wi"]), f32(d["Wf"])
    bq, bk, bv, bo = f32(d["bq"]), f32(d["bk"]), f32(d["bv"]), f32(d["bo"])
    bi, bf = f32(d["bi"]), f32(d["bf"])
    l1w, l1b = f32(d["ln1_w"]), f32(d["ln1_b"])
    l2w, l2b = f32(d["ln2_w"]), f32(d["ln2_b"])
    lew, leb = f32(d["ln_e_w"]), f32(d["ln_e_b"])

    # fold the producing LN's affine into consuming projections:
    # h = w*z + b  =>  h@W + c = z@(w[:,None]*W) + (b@W + c)
    w_pre = np.concatenate([lew[None], l2w[:-1]], axis=0)  # [L, Hd]
    b_pre = np.concatenate([leb[None], l2b[:-1]], axis=0)
    Wq_f = w_pre[:, :, None] * Wq
    Wk_f = w_pre[:, :, None] * Wk
    Wv_f = w_pre[:, :, None] * Wv
    bq_f = np.einsum("li,lio->lo", b_pre, Wq) + bq
    bk_f = np.einsum("li,lio->lo", b_pre, Wk) + bk
    bv_f = np.einsum("li,lio->lo", b_pre, Wv) + bv
    Wi_f = l1w[:, :, None] * Wi
    bi_f = np.einsum("li,lio->lo", l1b, Wi) + bi

    def colpack(x, n):  # [L, n*128] -> [L, 128, n]
        return np.ascontiguousarray(
            np.asarray(x, np.float32).reshape(L, n, 128).transpose(0, 2, 1)
        )

    par = np.concatenate(
        [
            colpack(bq_f, HC),
            colpack(bk_f, HC),
            colpack(bo, HC),
            colpack(bf, HC),
            colpack(l1w, HC),
            colpack(l1b, HC),
            colpack(l2w, HC),
            colpack(l2b, HC),
        ],
        axis=2,
    )
    # wi[l, oc, p, ic*128+j] = Wi[l, ic*128+p, oc*128+j]
    wi_r = np.ascontiguousarray(
        Wi_f.reshape(L, HC, 128, FC, 128)
        .transpose(0, 3, 2, 1, 4)
        .reshape(L, FC, 128, Hd)
        .astype(BF)
    )
    # wf[l, oc2, p, fc*128+j] = Wf[l, fc*128+p, oc2*128+j]
    wf_r = np.ascontiguousarray(
        Wf.reshape(L, FC, 128, HC, 128)
        .transpose(0, 3, 2, 1, 4)
        .reshape(L, HC, 128, FF)
        .astype(BF)
    )
    # epar[:, 0:HC] = lew cols, [:, HC:2HC] = leb cols
    epar = np.concatenate(
        [lew.reshape(HC, 128).T, leb.reshape(HC, 128).T], axis=1
    )
    return {
        "wq": Wq_f.astype(BF),
        "wk": Wk_f.astype(BF),
        "wv": Wv_f.astype(BF),
        "wo": Wo.astype(BF),
        "wi": wi_r,
        "wf": wf_r,
        "par": par,
        "bi": colpack(bi_f, FC),
        "bv": bv_f.astype(BF)[:, None, :],
        "wemb": f32(d["word_emb"]).astype(BF),
        "sel": _sel_matrix(cfg),
        "epar": np.ascontiguousarray(epar, np.float32),
    }


def prep_core_inputs(cfg: Cfg, core: int, d: dict, shared: dict) -> dict:
    TOK, TB, KB, HC, TPH = cfg.TOK, cfg.TB, cfg.KB, cfg.HC, cfg.TOK // 128
    b, hh = core // cfg.NHALF, core % cfg.NHALF
    ids = np.asarray(d["input_ids"], np.int32)[b, hh * TOK : (hh + 1) * TOK]
    mask = np.asarray(d["attention_mask"], np.float32)[b, 0, 0, :]
    pos = (
        np.asarray(d["pos_emb"], np.float32)[hh * TOK : (hh + 1) * TOK]
        + np.asarray(d["type_emb"], np.float32)[0][None, :]
    )
    m = dict(shared)
    m["pos"] = np.ascontiguousarray(pos).astype(BF)
    m["ids"] = np.ascontiguousarray(ids.reshape(TB, 128).T)
    # kb order: local half first, then remote half
    kb_order = [hh * TPH + j for j in range(TPH)] + [
        (1 - hh) * TPH + j for j in range(TPH)
    ]
    mask_cols = mask.reshape(KB, 128).T  # [128, KB] in global order
    m["mask"] = np.ascontiguousarray(mask_cols[:, kb_order])
    # indirect row indices for the remote half of kvk_out/kvv_out
    rem = 1 - hh
    ar = np.arange(128, dtype=np.int32)
    kcols = [rem * cfg.Hd + j * 128 + ar for j in range(HC)]
    vcols = [rem * TOK + j * 128 + ar for j in range(TPH)]
    m["kvidx"] = np.ascontiguousarray(np.stack(kcols + vcols, axis=1))
    return m


_CACHE: dict = {}


def kernel(**inputs) -> np.ndarray:
    cfg = Cfg()
    B = inputs["input_ids"].shape[0]
    if "nc" not in _CACHE:
        _CACHE["nc"] = build(cfg)
    nc = _CACHE["nc"]
    shared = prep_shared_inputs(cfg, inputs)
    in_maps = [prep_core_inputs(cfg, c, inputs, shared) for c in range(cfg.n_cores)]
    res = run_bass_kernel_spmd(nc, in_maps, core_ids=list(range(cfg.n_cores)))
    out = np.zeros((B, cfg.S, cfg.Hd), np.float32)
    for c in range(cfg.n_cores):
        b, hh = c // cfg.NHALF, c % cfg.NHALF
        out[b, hh * cfg.TOK : (hh + 1) * cfg.TOK, :] = np.asarray(
            res.results[c]["y"], np.float32
        )
    return out
